# revision 15
# baseline (speedup 1.0000x reference)
"""ColBERT pairwise scoring kernel for 8x TRN2 NeuronCores.

Computation (see problem reference):
    qn = l2norm(q, axis=-1); kn = l2norm(k, axis=-1)
    S[b,o,i,j] = qn[b,i,:]·kn[o,j,:], masked positions -> -inf
    s[b,o] = sum_i logsumexp_j(ALPHA*S)/ALPHA, nonfinite -> 0
    out = s / (sqrt(Lq*Lk)+1e-6) * min(exp(logit_scale), 100)

Sharding: candidate axis O is split across the 8 cores (16 o's per core);
q is replicated. Per core the kernel computes, for its o-shard:
    lse[o, bi] = sum_j exp(rk[j] * (k_raw[j,:]·qn[:,bi]))   (j = o*256..o*256+255)
where rk[j] = ALPHA/||k_j|| is folded into the ACT exp's per-partition scale
(orientation: j lives on PSUM partitions, bi on the free axis), the j-sum is
done on the tensor engine with indicator-column weights accumulating all 16
o-rows into one persistent PSUM tile, and the k-mask is handled by zeroing
masked k rows on the host (exp contributes exactly 1.0 there) and subtracting
the per-o masked count inside the final Ln's bias.

Since |ALPHA*S| <= 12, no max-subtraction is needed for a stable logsumexp.
"""

import math
import sys
from contextlib import ExitStack

import numpy as np

for _p in ("/opt/trn_rl_repo",):
    if _p not in sys.path:
        sys.path.insert(0, _p)

import concourse.bass as bass
import concourse.bacc as bacc
import concourse.tile as tile
from concourse import mybir
from concourse.bass_utils import run_bass_kernel_spmd

ALPHA = 12.0
B, Lq, O, Lk, D = 64, 32, 128, 256, 128
NCORES = 8
BI = B * Lq  # 2048 query rows, replicated on every core

F32 = mybir.dt.float32
AF = mybir.ActivationFunctionType
OP = mybir.AluOpType


def _bcast_ap(ap, parts):
    """Broadcast a [1, N] DRAM AP across `parts` partitions (step-0 AP)."""
    return bass.AP(tensor=ap.tensor, offset=ap.offset, ap=[[0, parts]] + list(ap.ap[1:]))


def emit_kernel(ctx, tc, q_d, k_d, nm_d, io_d, id_d, out_d, OL):
    """Emit the per-core program. OL = number of o's on this core."""
    nc = tc.nc
    KR = OL * Lk            # k rows on this core
    NKC = KR // 128         # k chunks (128 rows each)
    NQC = BI // 128         # q chunks = 16
    NIT = NKC               # main iterations, one per k chunk (= (o, jh))
    TG = 1024 if KR % 1024 == 0 else 512   # kt copy-group width (columns)
    NKG = KR // TG          # number of kt copy groups

    sing = ctx.enter_context(tc.tile_pool(name="sing", bufs=1))
    qnat = ctx.enter_context(tc.tile_pool(name="qnat", bufs=4))
    knat = ctx.enter_context(tc.tile_pool(name="knat", bufs=4))
    epool = ctx.enter_context(tc.tile_pool(name="epool", bufs=4))
    pm = ctx.enter_context(tc.tile_pool(name="pm", bufs=2, space="PSUM"))
    plse = ctx.enter_context(tc.tile_pool(name="plse", bufs=1, space="PSUM"))

    kt = sing.tile([128, KR], F32)      # transposed RAW k  [d, (o j)]
    qt = sing.tile([128, BI], F32)      # transposed NORMALIZED q [d, bi]
    nk = sing.tile([128, NKC], F32)     # per-k-chunk squared norms
    nq = sing.tile([128, NQC], F32)
    rk = sing.tile([128, NKC], F32)     # ALPHA / ||k row||
    rq = sing.tile([128, NQC], F32)     # 1 / ||q row||
    sqk = sing.tile([128, 128], F32)    # scratch for square outputs
    sqq = sing.tile([128, 128], F32)
    ecols = sing.tile([128, OL * OL], F32)
    id128 = sing.tile([128, 128], F32)
    negnm = sing.tile([128, 1], F32)
    loglse = sing.tile([OL, BI], F32)
    sres = sing.tile([OL, B], F32)
    bias_eps = sing.tile([128, 1], F32)   # 1e-30, keeps Ln(0) finite-input
    bias_lna = sing.tile([128, 1], F32)   # ln(ALPHA), folds alpha into rk
    nc.vector.memset(bias_eps, 1e-30)
    nc.vector.memset(bias_lna, math.log(ALPHA))

    # ---- constants in ----
    nc.sync.dma_start(out=id128, in_=id_d)
    nc.vector.memset(ecols, 0.0)
    for _o in range(OL):
        nc.vector.memset(ecols[:, _o * OL + _o:_o * OL + _o + 1], 1.0)
    nc.sync.dma_start(out=negnm, in_=nm_d)

    # ---- input loads: q quarters interleaved with k groups (512 rows each) ---
    qtiles = []
    ktiles = []
    NKLG = KR // 512
    for g in range(max(4, NKLG)):
        if g < 4:
            t = qnat.tile([128, 4, 128], F32, tag="qn")
            nc.sync.dma_start(
                out=t, in_=q_d[g * 512:(g + 1) * 512, :].rearrange("(c p) d -> p c d", p=128)
            )
            qtiles.append(t)
        if g < NKLG:
            t = knat.tile([128, 4, 128], F32, tag="kn")
            nc.sync.dma_start(
                out=t, in_=k_d[g * 512:(g + 1) * 512, :].rearrange("(c p) d -> p c d", p=128)
            )
            ktiles.append(t)

    # ---- q pipeline: norms -> rq -> scale -> transpose -> qt (per quarter) ----
    # DVE: per-chunk squared-norm accumulate; ACT: rq = exp(-0.5*ln(n2+eps)).
    for g in range(4):
        for c in range(4):
            cc = 4 * g + c
            nc.vector.tensor_mul(sqq, qtiles[g][:, c, :], qtiles[g][:, c, :])
            nc.vector.reduce_sum(out=nq[:, cc:cc + 1], in_=sqq,
                                 axis=mybir.AxisListType.X)
        nc.scalar.activation(out=rq[:, g * 4:(g + 1) * 4], in_=nq[:, g * 4:(g + 1) * 4],
                             func=AF.Sqrt, bias=bias_eps[:, 0:1], scale=1.0)
        nc.vector.reciprocal(out=rq[:, g * 4:(g + 1) * 4], in_=rq[:, g * 4:(g + 1) * 4])
        for c in range(4):
            cc = 4 * g + c
            nc.vector.tensor_scalar(
                out=qtiles[g][:, c, :], in0=qtiles[g][:, c, :],
                scalar1=rq[:, cc:cc + 1], scalar2=None, op0=OP.mult,
            )

    # ---- k norms (DVE) + rk (ACT, two batches; second emitted mid-loop) ----
    def emit_k_norms(c0, c1):
        for cc in range(c0, c1):
            nc.vector.tensor_mul(sqk, ktiles[cc // 4][:, cc % 4, :],
                                 ktiles[cc // 4][:, cc % 4, :])
            nc.vector.reduce_sum(out=nk[:, cc:cc + 1], in_=sqk,
                                 axis=mybir.AxisListType.X)

    def emit_rk(c0, c1):
        nc.scalar.activation(out=rk[:, c0:c1], in_=nk[:, c0:c1],
                             func=AF.Sqrt, bias=bias_eps[:, 0:1], scale=1.0)
        nc.vector.reciprocal(out=rk[:, c0:c1], in_=rk[:, c0:c1])
        nc.vector.tensor_scalar_mul(rk[:, c0:c1], rk[:, c0:c1], float(ALPHA))

    emit_k_norms(0, NKC // 2)
    emit_rk(0, NKC // 2)
    emit_k_norms(NKC // 2, NKC)

    # ---- transposes: PE transpose [128,128] blocks into PSUM, DVE copy out ---
    def transpose_group(dst, dst_off, src_tiles, src_chunk0, nchunks):
        """Transpose `nchunks` natural chunks into dst[:, dst_off:dst_off+128*n]."""
        for i in range(nchunks):
            cc = src_chunk0 + i
            pt = pm.tile([128, 128], F32, tag="mm")
            nc.tensor.transpose(
                out=pt, in_=src_tiles[cc // 4][:, cc % 4, :], identity=id128)
            nc.vector.tensor_copy(
                out=dst[:, dst_off + i * 128: dst_off + (i + 1) * 128], in_=pt)

    # k groups 0..1 first (needed by early mains), then q, then rest of k later
    early_kg = min(2, NKG)
    for g in range(early_kg):
        transpose_group(kt, g * TG, ktiles, g * (TG // 128), TG // 128)
    for g in range(4):
        transpose_group(qt, g * 512, qtiles, g * 4, 4)

    # ---- main loop: software-pipelined matmul -> exp -> reduce-matmul ----
    lse = plse.tile([OL, BI], F32)
    et = {}
    for it in range(NIT + 1):
        if it == 4 and NKG > early_kg:
            for g in range(early_kg, NKG):
                transpose_group(kt, g * TG, ktiles, g * (TG // 128), TG // 128)
        if it == NIT // 2:
            emit_rk(NKC // 2, NKC)
        if it < NIT:
            o = it // 2
            ts = []
            es = []
            for h in range(2):
                T = pm.tile([128, 1024], F32, tag="mm")
                for s2 in range(2):
                    nc.tensor.matmul(
                        out=T[:, s2 * 512:(s2 + 1) * 512],
                        lhsT=kt[:, it * 128:(it + 1) * 128],
                        rhs=qt[:, h * 1024 + s2 * 512: h * 1024 + (s2 + 1) * 512],
                        start=True, stop=True,
                    )
                ts.append(T)
            for h in range(2):
                e = epool.tile([128, 1024], F32, tag="e")
                nc.scalar.activation(out=e, in_=ts[h], func=AF.Exp,
                                     bias=0.0, scale=rk[:, it:it + 1])
                es.append(e)
            et[it] = es
        if it > 0:
            p = it - 1
            o_p = p // 2
            for h, e in enumerate(et.pop(p)):
                for s2 in range(2):
                    nc.tensor.matmul(
                        out=lse[0:OL, h * 1024 + s2 * 512: h * 1024 + (s2 + 1) * 512],
                        lhsT=ecols[:, o_p * OL:(o_p + 1) * OL],
                        rhs=e[:, s2 * 512:(s2 + 1) * 512],
                        start=(p == 0), stop=(p == NIT - 1),
                    )

    # ---- tail: log(sum - n_masked), sum over Lq, store ----
    nc.scalar.activation(out=loglse, in_=lse[0:OL, :], func=AF.Ln,
                         bias=negnm[0:OL, 0:1], scale=1.0)
    nc.vector.tensor_reduce(
        out=sres, in_=loglse.rearrange("p (b i) -> p b i", i=Lq),
        axis=mybir.AxisListType.X, op=OP.add,
    )
    nc.sync.dma_start(out=out_d, in_=sres)


def build_program(OL):
    KR = OL * Lk
    nc = bacc.Bacc("TRN2", target_bir_lowering=False, debug=False,
                   enable_asserts=False, num_devices=NCORES)
    q_d = nc.dram_tensor("q_in", [BI, D], F32, kind="ExternalInput").ap()
    k_d = nc.dram_tensor("k_in", [KR, D], F32, kind="ExternalInput").ap()
    nm_d = nc.dram_tensor("negnm", [128, 1], F32, kind="ExternalInput").ap()
    id_d = nc.dram_tensor("id128", [128, 128], F32, kind="ExternalInput").ap()
    out_d = nc.dram_tensor("outp", [OL, B], F32, kind="ExternalOutput").ap()

    with tile.TileContext(nc) as tc, ExitStack() as ctx:
        emit_kernel(ctx, tc, q_d, k_d, nm_d, None, id_d, out_d, OL)
    nc.compile()
    return nc


def make_in_maps(q, k, k_mask, OL, ncores):
    """Host-side shard prep. Returns per-core input dicts."""
    qf = np.ascontiguousarray(q.reshape(BI, D), dtype=np.float32)
    kz = np.ascontiguousarray(k, dtype=np.float32).copy()
    kz[k_mask.astype(bool)] = 0.0
    nmask = k_mask.astype(bool).sum(axis=1).astype(np.float32)  # [O]
    id128 = np.eye(128, dtype=np.float32)
    in_maps = []
    for c in range(ncores):
        osl = slice(c * OL, (c + 1) * OL)
        in_maps.append({
            "q_in": qf,
            "k_in": np.ascontiguousarray(kz[osl].reshape(OL * Lk, D)),
            "negnm": np.ascontiguousarray(
                np.pad(-nmask[osl], (0, 128 - OL)).reshape(128, 1)),
            "id128": id128,
        })
    return in_maps


def postprocess(per_core_out, q_mask, k_mask, logit_scale, OL, ncores):
    """Gather per-core [OL, B] results into the final [B, O] output."""
    s = np.empty((B, ncores * OL), dtype=np.float32)
    for c in range(ncores):
        s[:, c * OL:(c + 1) * OL] = per_core_out[c].T
    coef = min(math.exp(float(logit_scale)), 100.0) / (
        ALPHA * (math.sqrt(Lq * Lk) + 1e-06))
    s = s * np.float32(coef)
    # rows with any masked query token are -inf in the reference -> zeroed
    s[np.asarray(q_mask).astype(bool).any(axis=1), :] = 0.0
    # fully-masked candidates are -inf in the reference -> zeroed
    s[:, np.asarray(k_mask).astype(bool).all(axis=1)] = 0.0
    s = np.where(np.isfinite(s), s, 0.0).astype(np.float32)
    return s


_CACHED_NC = None


def kernel(q, k, q_mask, k_mask, logit_scale):
    global _CACHED_NC
    OL = O // NCORES
    if _CACHED_NC is None:
        _CACHED_NC = build_program(OL)
    in_maps = make_in_maps(np.asarray(q), np.asarray(k), np.asarray(k_mask), OL, NCORES)
    res = run_bass_kernel_spmd(_CACHED_NC, in_maps, list(range(NCORES)))
    outs = [np.asarray(res.results[c]["outp"]) for c in range(NCORES)]
    return postprocess(outs, q_mask, k_mask, logit_scale, OL, NCORES)
